# revision 74
# baseline (speedup 1.0000x reference)
"""Trainium2 Bass kernel for nn_BasicTransformerBlock (self-attn + cross-attn + GEGLU).

Sharding: data-parallel over the 2048 tokens (256 per core, 8 cores).
K/V for self-attention are computed on OWN tokens only and exchanged via
two AllGathers (K first so scores can start earlier).

LayerNorm is never materialized: per-token stats (mu, rstd) are computed
from the f32 residual stream, the input is pre-scaled once
(xs = rstd * x, bf16), and the -mu*colsum(W) correction enters each
projection's PSUM accumulation group as one extra contract-1 matmul
(lhsT = -colsum(W) row, rhs = rstd*mu row).  LN gains are folded into
the weights host-side.  This removes LN from the critical path three
times and lets the K projection -> AllGather start ~10us into the
kernel.

Softmax: scores keys-on-partitions; head pairs at PE row groups 0:64 /
64:128 run concurrently; exp on ScalarE over [128,1024] PSUM spans;
denominators from an appended ones-column in V; one wide reciprocal for
all 16 heads, broadcast via PE rank-1 matmuls.

FFN: GEGLU and ffout matmuls interleave; ffout accumulates all 32
contraction tiles directly in persistent PSUM (no spills), residual adds
fused at the end.
"""

import numpy as np
import ml_dtypes

import concourse.bass as bass
import concourse.mybir as mybir
import concourse.tile as tile
from concourse import bacc
from concourse.bass_utils import run_bass_kernel_spmd

F32 = mybir.dt.float32
F32R = mybir.dt.float32r
BF16 = mybir.dt.bfloat16
FP8 = mybir.dt.float8e4
AF = mybir.ActivationFunctionType
OP = mybir.AluOpType

SX = 16.0                 # fp8 scale for K/Q/V (1/SX folded into wo1/wo2)

P = 128
N, D = 2048, 1024
H, DH = 16, 64
CN, CD = 77, 768
FF = 4096
EPS = 1e-5
SCALE = DH ** -0.5
NCORES = 8
TO = N // NCORES          # 256 tokens owned per core
DT = D // P               # 8 feature tiles
CT = CD // P              # 6 context-feature tiles
NKT = N // P              # 16 key tiles
FT = FF // P              # 32 ffn-inner tiles


def build():
    nc = bacc.Bacc(num_devices=NCORES)

    xoT = nc.dram_tensor("xoT", [D, TO], F32R, kind="ExternalInput")
    xobT = nc.dram_tensor("xobT", [D, TO], BF16, kind="ExternalInput")
    ctxT = nc.dram_tensor("ctxT", [CD, CN], BF16, kind="ExternalInput")
    wq1T = nc.dram_tensor("wq1T", [D, D], BF16, kind="ExternalInput")
    wk1T = nc.dram_tensor("wk1T", [D, D], BF16, kind="ExternalInput")
    wv1T = nc.dram_tensor("wv1T", [D, D], BF16, kind="ExternalInput")
    wo1T = nc.dram_tensor("wo1T", [D, D], BF16, kind="ExternalInput")
    wq2T = nc.dram_tensor("wq2T", [D, D], BF16, kind="ExternalInput")
    wk2T = nc.dram_tensor("wk2T", [CD, D], BF16, kind="ExternalInput")
    wv2T = nc.dram_tensor("wv2T", [CD, D], BF16, kind="ExternalInput")
    wo2T = nc.dram_tensor("wo2T", [D, D], BF16, kind="ExternalInput")
    wgT = nc.dram_tensor("wgT", [D, 2 * FF], BF16, kind="ExternalInput")
    wfT = nc.dram_tensor("wfT", [FF, D], BF16, kind="ExternalInput")
    # negated column sums for the mean fixup
    nsk1 = nc.dram_tensor("nsk1", [1, D], BF16, kind="ExternalInput")
    nsq1 = nc.dram_tensor("nsq1", [1, D], BF16, kind="ExternalInput")
    nsv1 = nc.dram_tensor("nsv1", [1, D], BF16, kind="ExternalInput")
    nsq2 = nc.dram_tensor("nsq2", [1, D], BF16, kind="ExternalInput")
    onesc = nc.dram_tensor("onesc", [P, 1], F32R, kind="ExternalInput")
    onesr = nc.dram_tensor("onesr", [1, P], F32R, kind="ExternalInput")
    yT = nc.dram_tensor("yT", [D, TO], F32R, kind="ExternalOutput")

    xoT_v = xoT.rearrange("(dt p) t -> dt p t", p=P)
    xobT_v = xobT.rearrange("(dt p) t -> dt p t", p=P)
    ctxT_v = ctxT.rearrange("(ct p) t -> ct p t", p=P)
    yT_v = yT.rearrange("(dt p) t -> p dt t", p=P)

    def wview(w):
        return w.rearrange("(it p) o -> p it o", p=P)

    with tile.TileContext(nc) as tc:
        with tc.tile_pool(name="consts", bufs=1) as cpool, \
             tc.tile_pool(name="pers", bufs=1) as pers, \
             tc.tile_pool(name="wmain", bufs=1) as wmain, \
             tc.tile_pool(name="stat", bufs=1) as statp, \
             tc.tile_pool(name="agdram", bufs=1, space="DRAM") as agd:

            ones_col = cpool.tile([P, 1], F32R)
            nc.sync.dma_start(ones_col, onesc[:])
            ones_col_bf = cpool.tile([P, 1], BF16)
            nc.vector.memset(ones_col_bf, 1.0)
            ones_row = cpool.tile([1, P], F32R)
            nc.sync.dma_start(ones_row, onesr[:])
            ones2_f = cpool.tile([P, 64], F32)
            nc.vector.memset(ones2_f, 1.0)
            ones2 = ones2_f.bitcast(F32R)
            eps_t = cpool.tile([1, 1], F32)
            nc.vector.memset(eps_t, EPS)

            ns_sb = {}
            for nm, t in (("k1", nsk1), ("q1", nsq1), ("v1", nsv1),
                          ("q2", nsq2)):
                bt = cpool.tile([1, D], BF16, tag=f"ns_{nm}")
                nc.scalar.dma_start(bt, t[:])
                ns_sb[nm] = bt

            def stats(src_of, tag, bf=False, raw_mu=False):
                """Per-token mean/rstd of feature-major data.

                Returns (mu_bf [1,TO] bf16 (= rstd*mu, or raw mu if raw_mu),
                         rstd_sb [P,TO] f32 broadcast,
                         rstd_row [1,TO] f32r)."""
                inv_d = 1.0 / D
                oc_ = ones_col_bf if bf else ones_col
                with tc.tile_pool(name=f"st_{tag}", bufs=1, space="PSUM") as sp:
                    sum_ps = sp.tile([1, TO], F32, tag="s_sum")
                    sumsq_ps = sp.tile([1, TO], F32, tag="s_sumsq")
                    for dt in range(DT):
                        nc.tensor.matmul(sum_ps, oc_, src_of(dt),
                                         start=(dt == 0), stop=(dt == DT - 1))
                    for dt in range(DT):
                        sq_t = statp.tile([P, TO], F32R, tag="st_sq", bufs=3)
                        src = src_of(dt)
                        srcf = src if bf else src.bitcast(F32)
                        if dt % 2 == 0:
                            nc.scalar.activation(sq_t, srcf, AF.Square)
                        else:
                            with nc.allow_low_precision("f32r square feed"):
                                nc.gpsimd.tensor_mul(out=sq_t, in0=srcf,
                                                     in1=srcf)
                        nc.tensor.matmul(sumsq_ps, ones_col, sq_t,
                                         start=(dt == 0), stop=(dt == DT - 1))
                    mu_row = statp.tile([1, TO], F32R, tag="st_mu", bufs=2)
                    nc.scalar.mul(out=mu_row, in_=sum_ps, mul=inv_d)
                    var_row = statp.tile([1, TO], F32, tag="st_var", bufs=2)
                    nc.scalar.mul(out=var_row, in_=sumsq_ps, mul=inv_d)
                    musq = statp.tile([1, TO], F32, tag="st_musq", bufs=2)
                    nc.vector.tensor_mul(out=musq, in0=mu_row.bitcast(F32),
                                         in1=mu_row.bitcast(F32))
                    nc.vector.tensor_tensor(out=var_row, in0=var_row,
                                            in1=musq, op=OP.subtract)
                    nc.scalar.activation(var_row, var_row, AF.Sqrt,
                                         bias=eps_t)
                    rstd_row = statp.tile([1, TO], F32R, tag="st_rstd",
                                          bufs=2)
                    with nc.allow_low_precision("f32r keeps full fp32 bits"):
                        nc.vector.reciprocal(rstd_row, var_row)
                    mu_bf = statp.tile([1, TO], BF16, tag="st_mus", bufs=2)
                    if raw_mu:
                        nc.vector.tensor_copy(out=mu_bf,
                                              in_=mu_row.bitcast(F32))
                    else:
                        nc.vector.tensor_mul(out=mu_bf,
                                             in0=mu_row.bitcast(F32),
                                             in1=rstd_row.bitcast(F32))
                return mu_bf, rstd_row, mu_row

            def bcast_rstd(pp, rstd_row):
                """Broadcast rstd to [P,TO] f32 in SBUF.  Emitted LATE so
                the PE-queue matmul doesn't head-of-line-block projection
                matmuls behind the stats chain."""
                rstd_ps = pp.tile([P, TO], F32, tag="s_bc", bufs=1)
                nc.tensor.matmul(rstd_ps, ones_row, rstd_row,
                                 start=True, stop=True)
                rstd_sb = statp.tile([P, TO], F32, tag="st_bc", bufs=2)
                nc.vector.tensor_copy(out=rstd_sb, in_=rstd_ps)
                return rstd_sb

            def prescale(xs, rstd_sb):
                """xs[:,dt,:] = bf16(rstd * x)."""
                for dt in range(DT):
                    eng = nc.gpsimd if dt % 2 else nc.vector
                    eng.tensor_tensor(
                        out=xs[:, dt, :],
                        in0=x_ownT[:, dt, :].bitcast(F32),
                        in1=rstd_sb, op=OP.mult)

            def proj(pp, w_sb, xs, out_cb, fixup=None, n_in=DT, n_tok=TO,
                     tag="pp256", bufs=2, pre_evac=None):
                """psum[oc] = W.T @ xs (+ mean fixup).

                With a fixup, ALL main matmuls are emitted before any
                fixup matmul (and pre_evac before any evacuation) so the
                PE FIFO isn't head-of-line-blocked on the stats chain."""
                if fixup is None:
                    for oc in range(DT):
                        ps = pp.tile([P, n_tok], F32, tag=tag, bufs=bufs)
                        for it in range(n_in):
                            nc.tensor.matmul(
                                ps, w_sb[:, it, oc * P:(oc + 1) * P],
                                xs[:, it, :],
                                start=(it == 0), stop=(it == n_in - 1))
                        out_cb(oc, ps)
                    return
                # oc pairs share a 2KB PSUM bank (bank-granular alloc):
                # even oc's first matmul clears the bank; odd oc's first
                # write lands on cleared has_written bits and overwrites.
                ns, mu_s = fixup
                pss = []
                for oc2 in range(DT // 2):
                    ps = pp.tile([P, 2, n_tok], F32, tag=tag, bufs=DT // 2)
                    for k in range(2):
                        oc = 2 * oc2 + k
                        for it in range(n_in):
                            nc.tensor.matmul(
                                ps[:, k, :],
                                w_sb[:, it, oc * P:(oc + 1) * P],
                                xs[:, it, :],
                                start=(k == 0 and it == 0), stop=False,
                                skip_group_check=(k == 1 and it == 0))
                    pss.append(ps)
                for oc in range(DT):
                    nc.tensor.matmul(
                        pss[oc // 2][:, oc % 2, :],
                        ns[0:1, oc * P:(oc + 1) * P],
                        mu_s, start=False, stop=True)
                if pre_evac is not None:
                    pre_evac()
                for oc in range(DT):
                    out_cb(oc, pss[oc // 2][:, oc % 2, :])

            xobT_w = xobT.rearrange("(dt p) t -> p dt t", p=P)
            xb = pers.tile([P, DT, TO], BF16)          # raw x, bf16 (host)
            nc.sync.dma_start(xb[:, 0:4, :], xobT_w[:, 0:4, :])
            nc.sync.dma_start(xb[:, 4:8, :], xobT_w[:, 4:8, :])
            x_ownT = pers.tile([P, DT, TO], F32R)      # residual stream (own)
            for dt in range(DT):
                nc.gpsimd.dma_start(x_ownT[:, dt, :], xoT_v[dt])

            # cross-attn K2/V2 depend only on the context
            K2_sb = pers.tile([P, DT, CN], FP8)
            V2_sb = pers.tile([P, H, 65], FP8)

            # ========== attn1 scope: phases A-D ==========
            with tc.tile_pool(name="c1", bufs=1) as c1:
                O_sb = c1.tile([P, DT, TO], BF16)
                K_sb = c1.tile([P, DT, N], FP8)
                ctx_sb = c1.tile([P, CT, CN], BF16)
                wk2_sb = c1.tile([P, CT, D], BF16)
                wv2_sb = c1.tile([P, CT, D], BF16)
                V_sb = c1.tile([P, NKT, H, 65], FP8)
                Q_sb = c1.tile([P, DT, TO], FP8)

                # ----- Phase A: stats + K/V own + AllGather + Q1 -----
                # K/Q/V project the RAW bf16 x; the LN mean enters as one
                # contract-1 matmul (-colsum(W) x raw mu) appended to each
                # PSUM group, and rstd (and the fp8 x16 scale) applies on
                # the PSUM->SBUF evacuation.  K starts as soon as the first
                # wk1 chunk and x tile land.
                scopeA = nc.enter_named_scope("phA_qkv", False)
                wk1_sb = wmain.tile([P, DT, D], BF16, tag="w2m", bufs=2)
                wk1_v = wview(wk1T)
                nc.sync.dma_start(wk1_sb[:, 0:4, :], wk1_v[:, 0:4, :])
                nc.sync.dma_start(wk1_sb[:, 4:8, :], wk1_v[:, 4:8, :])
                wv1_sb = wmain.tile([P, DT, D], BF16, tag="w2m", bufs=2)
                wv1_v = wview(wv1T)
                nc.scalar.dma_start(wv1_sb[:, 0:4, :], wv1_v[:, 0:4, :])
                nc.scalar.dma_start(wv1_sb[:, 4:8, :], wv1_v[:, 4:8, :])
                wq1_sb = wmain.tile([P, DT, D], BF16, tag="w2m", bufs=2)
                nc.scalar.dma_start(wq1_sb, wview(wq1T))

                kown_sb = c1.tile([P, DT, TO], FP8)
                vown_sb = c1.tile([P, 2, D], FP8)

                mu1, rstd1_row, _ = stats(
                    lambda dt: xb[:, dt, :], "ln1", bf=True, raw_mu=True)

                with tc.tile_pool(name="aps_", bufs=2, space="PSUM") as app:
                    hold = {}
                    rcol_sb = []

                    def pre1():
                        hold["rstd"] = bcast_rstd(app, rstd1_row)
                        # per-token-chunk rstd as a [128,1] column (for
                        # V's token-major output fixup)
                        for ktl in range(2):
                            r_ps = app.tile([P, 2], F32, tag="rcol",
                                            bufs=1)
                            nc.tensor.matmul(
                                r_ps,
                                rstd1_row[0:1, ktl * P:(ktl + 1) * P],
                                ones_row[0:1, 0:2], start=True, stop=True)
                            rc = statp.tile([P, 2], F32, tag="st_rcol",
                                            bufs=2)
                            nc.vector.tensor_copy(out=rc, in_=r_ps)
                            rcol_sb.append(rc)

                    # K own (feature-major [D, TO], fp8 = SX*K);
                    # partition-major bounce layout keeps per-partition
                    # lines contiguous on the DRAM side.
                    kb_in = agd.tile([D * TO], FP8)
                    kb_inv = kb_in[:].rearrange(
                        "(p dt t) -> p dt t", dt=DT, t=TO)
                    vb_in = agd.tile([TO * D], FP8)
                    vb_inv = vb_in[:].rearrange(
                        "(p ktl d) -> p ktl d", ktl=2, d=D)
                    proj(
                        app, wk1_sb, xb,
                        lambda oc, ps: nc.vector.scalar_tensor_tensor(
                            out=kown_sb[:, oc, :], in0=ps, scalar=SX,
                            in1=hold["rstd"], op0=OP.mult, op1=OP.mult),
                        fixup=(ns_sb["k1"], mu1), tag="ppk", bufs=8,
                        pre_evac=pre1)
                    nc.sync.dma_start(kb_inv, kown_sb[:])
                    kb_out = agd.tile([NCORES * D * TO], FP8,
                                      addr_space="Shared")
                    nc.gpsimd.collective_compute(
                        "AllGather", OP.bypass,
                        replica_groups=[list(range(NCORES))],
                        ins=[kb_in[:]], outs=[kb_out[:]])

                    # V own (keys-on-partitions [TO, D], fp8 = SX*V); runs
                    # after K, so mu1/rcol are already available and the
                    # per-tile fixup doesn't block the PE queue.
                    for ktl in range(2):
                        for hc in range(2):
                            v_ps = app.tile([P, 512], F32, tag="pp512",
                                            bufs=2)
                            for it in range(DT):
                                nc.tensor.matmul(
                                    v_ps,
                                    xb[:, it, ktl * P:(ktl + 1) * P],
                                    wv1_sb[:, it, hc * 512:(hc + 1) * 512],
                                    start=(it == 0), stop=False)
                            nc.tensor.matmul(
                                v_ps, mu1[0:1, ktl * P:(ktl + 1) * P],
                                ns_sb["v1"][0:1, hc * 512:(hc + 1) * 512],
                                start=False, stop=True)
                            nc.vector.tensor_scalar(
                                out=vown_sb[:, ktl, hc * 512:(hc + 1) * 512],
                                in0=v_ps, scalar1=rcol_sb[ktl][:, 0:1],
                                scalar2=SX, op0=OP.mult, op1=OP.mult)
                    nc.scalar.dma_start(vb_inv, vown_sb[:])
                    vb_out = agd.tile([NCORES * TO * D], FP8,
                                      addr_space="Shared")
                    nc.gpsimd.collective_compute(
                        "AllGather", OP.bypass,
                        replica_groups=[list(range(NCORES))],
                        ins=[vb_in[:]], outs=[vb_out[:]])

                    # queue the context/cross-attn weight loads now
                    for ct in range(CT):
                        nc.scalar.dma_start(ctx_sb[:, ct, :], ctxT_v[ct])
                    nc.scalar.dma_start(wk2_sb, wview(wk2T))
                    nc.scalar.dma_start(wv2_sb, wview(wv2T))

                    # Q1 (overlaps the AllGathers), fp8 = SX*Q
                    proj(
                        app, wq1_sb, xb,
                        lambda oc, ps: nc.vector.scalar_tensor_tensor(
                            out=Q_sb[:, oc, :], in0=ps, scalar=SX,
                            in1=hold["rstd"], op0=OP.mult, op1=OP.mult),
                        fixup=(ns_sb["q1"], mu1), tag="ppk", bufs=8)
                nc.leave_named_scope("phA_qkv", scopeA[0], False)

                # ----- Phase B: K2/V2 (context) + gather-in K/V -----
                scopeB = nc.enter_named_scope("phB_kv2", False)
                with tc.tile_pool(name="projps2", bufs=2, space="PSUM") as pp:
                    for oc in range(DT):
                        k_ps = pp.tile([P, CN], F32, tag="ppsm", bufs=2)
                        for it in range(CT):
                            nc.tensor.matmul(
                                k_ps, wk2_sb[:, it, oc * P:(oc + 1) * P],
                                ctx_sb[:, it, :],
                                start=(it == 0), stop=(it == CT - 1))
                        nc.scalar.mul(out=K2_sb[:, oc, :], in_=k_ps,
                                      mul=SX)
                    nc.vector.memset(V2_sb[:, :, 64:65], 1.0)
                    for hc in range(2):
                        v_ps = pp.tile([CN, 512], F32, tag="ppsm", bufs=2)
                        for it in range(CT):
                            nc.tensor.matmul(
                                v_ps, ctx_sb[:, it, :],
                                wv2_sb[:, it, hc * 512:(hc + 1) * 512],
                                start=(it == 0), stop=(it == CT - 1))
                        nc.scalar.mul(
                            out=V2_sb[0:CN, hc * 8:(hc + 1) * 8, 0:64],
                            in_=v_ps.rearrange("p (h d) -> p h d", d=64),
                            mul=SX)

                    # gather-in K (sync+scalar queues) and V (gpsimd)
                    kb_ov = kb_out.rearrange("(c p dt t) -> c p dt t",
                                             p=P, dt=DT, t=TO)
                    nc.vector.memset(V_sb[:, :, :, 64:65], 1.0)
                    for c in range(NCORES):
                        eng = nc.sync if c % 2 == 0 else nc.scalar
                        eng.dma_start(
                            K_sb[:, :, c * TO:(c + 1) * TO], kb_ov[c])
                    vb_ov = vb_out.rearrange(
                        "(c p ktl h dh) -> c p ktl h dh",
                        p=P, ktl=2, h=H, dh=DH)
                    # one coalesced gather per core (per-DMA fixed cost
                    # dominates small transfers), split over two queues
                    for c in range(NCORES):
                        eng = nc.gpsimd if c % 2 == 0 else nc.sync
                        eng.dma_start(
                            V_sb[:, c * 2:c * 2 + 2, :, 0:64],
                            vb_ov[c])
                nc.leave_named_scope("phB_kv2", scopeB[0], False)

                # ----- Phase C: self-attention, head pairs -----
                scopeC = nc.enter_named_scope("phC_attn", False)
                wo1_sb = wmain.tile([P, DT, D], BF16, tag="w2m", bufs=2)
                nc.scalar.dma_start(wo1_sb, wview(wo1T))
                Ou_sb = c1.tile([P, DT, TO], F32)    # unnormalized AV outputs
                # softmax denominators: head h at partition (h%4)*32, free
                # slot h//4, so one wide reciprocal covers all 16 heads
                den_sb = c1.tile([P, 4, TO], F32)
                with tc.tile_pool(name="cps", bufs=1, space="PSUM") as cps, \
                     tc.tile_pool(name="aps", bufs=1, space="PSUM") as apsum, \
                     tc.tile_pool(name="asb", bufs=1) as asb:
                    for j in range(DT):              # head pair j
                        o_ps0 = apsum.tile([65, TO], F32, tag="o_ps", bufs=4)
                        o_ps1 = apsum.tile([65, TO], F32, tag="o_ps", bufs=4)
                        for g in range(8):           # 2 key tiles per group
                            s_ps = cps.tile([P, 4, TO], F32, tag="s_ps",
                                            bufs=2)
                            for half in range(2):
                                kt = g * 2 + half
                                # h0 at rows 0:64 (rg01), h1 at rows 64:128
                                # (rg23): issued interleaved so the PE can
                                # run them concurrently.
                                for hh in range(2):
                                    r0 = hh * 64
                                    nc.tensor.matmul(
                                        s_ps[:, 2 * hh + half, :],
                                        K_sb[r0:r0 + 64, j,
                                             kt * P:(kt + 1) * P],
                                        Q_sb[r0:r0 + 64, j, :],
                                        start=(half == 0), stop=True,
                                        skip_group_check=(half == 1))
                            e_t = asb.tile([P, 4, TO], FP8, tag="e_t",
                                           bufs=10)
                            nc.scalar.activation(e_t, s_ps, AF.Exp,
                                                 scale=SCALE / (SX * SX))
                            for half in range(2):
                                kt = g * 2 + half
                                nc.tensor.matmul(
                                    o_ps0, V_sb[:, kt, 2 * j, :],
                                    e_t[:, half, :],
                                    start=(g == 0 and half == 0),
                                    stop=(g == 7 and half == 1))
                                nc.tensor.matmul(
                                    o_ps1, V_sb[:, kt, 2 * j + 1, :],
                                    e_t[:, 2 + half, :],
                                    start=(g == 0 and half == 0),
                                    stop=(g == 7 and half == 1))
                        # stage unnormalized output + denominators
                        h0, h1 = 2 * j, 2 * j + 1
                        nc.vector.tensor_copy(out=Ou_sb[0:64, j, :],
                                              in_=o_ps0[0:64, :])
                        nc.vector.tensor_copy(
                            out=den_sb[(h0 % 4) * 32:(h0 % 4) * 32 + 1,
                                       h0 // 4, :],
                            in_=o_ps0[64:65, :])
                        nc.vector.tensor_copy(out=Ou_sb[64:128, j, :],
                                              in_=o_ps1[0:64, :])
                        nc.vector.tensor_copy(
                            out=den_sb[(h1 % 4) * 32:(h1 % 4) * 32 + 1,
                                       h1 // 4, :],
                            in_=o_ps1[64:65, :])
                    # batched softmax normalization
                    rec_sb = asb.tile([P, 4, TO], F32R, tag="rec", bufs=1)
                    rscr = asb.tile([P, 4, TO], F32, tag="rscr", bufs=1)
                    rec_f = asb.tile([P, 4, TO], F32, tag="recf", bufs=1)
                    nc.vector.reciprocal_approx_accurate(rec_f, den_sb, rscr)
                    with nc.allow_low_precision("f32r round of recip"):
                        nc.vector.tensor_copy(out=rec_sb, in_=rec_f)
                    for h in range(H):
                        j, r0 = h >> 1, (h & 1) * 64
                        b, s = (h % 4) * 32, h // 4
                        if b == 96:
                            # PE operand base must be 0/32/64: stage at 0
                            rfix = asb.tile([1, TO], F32R, tag="rfix",
                                            bufs=4)
                            nc.vector.tensor_copy(out=rfix,
                                                  in_=rec_sb[96:97, s, :])
                            lhs_ap, rhs_ap = ones2[0:1, :], rfix
                        else:
                            lhs_ap = ones2[b:b + 1, :]
                            rhs_ap = rec_sb[b:b + 1, s, :]
                        r_ps = apsum.tile([64, TO], F32, tag="o_ps", bufs=4)
                        nc.tensor.matmul(r_ps, lhs_ap, rhs_ap,
                                         start=True, stop=True)
                        nc.vector.tensor_tensor(
                            out=O_sb[r0:r0 + 64, j, :],
                            in0=Ou_sb[r0:r0 + 64, j, :],
                            in1=r_ps, op=OP.mult)
                nc.leave_named_scope("phC_attn", scopeC[0], False)

                # ----- Phase D: attn1 out-proj + residual -----
                scopeD = nc.enter_named_scope("phD_oproj", False)
                with tc.tile_pool(name="dps", bufs=3, space="PSUM") as pp:
                    def add_residual(oc, ps):
                        nc.vector.tensor_tensor(
                            out=x_ownT[:, oc, :],
                            in0=x_ownT[:, oc, :].bitcast(F32),
                            in1=ps, op=OP.add)

                    proj(pp, wo1_sb, O_sb, add_residual)
                nc.leave_named_scope("phD_oproj", scopeD[0], False)

            # ========== attn2 scope: phase E ==========
            scopeE = nc.enter_named_scope("phE_xattn", False)
            with tc.tile_pool(name="ce", bufs=1) as ce:
                xs2 = ce.tile([P, DT, TO], BF16)
                Q2_sb = ce.tile([P, DT, TO], FP8)
                O2_sb = ce.tile([P, DT, TO], BF16)
                Ou2_sb = ce.tile([P, DT, TO], F32)
                den2_sb = ce.tile([P, 4, TO], F32)

                # x1 cast to bf16; Q2 projects the raw cast and fixes the
                # mean/rstd on evacuation, so the matmuls overlap stats2
                mu2, rstd2_row, _ = stats(lambda dt: x_ownT[:, dt, :], "ln2",
                                          raw_mu=True)
                for dt in range(DT):
                    eng = nc.gpsimd if dt % 2 else nc.vector
                    eng.tensor_copy(out=xs2[:, dt, :],
                                    in_=x_ownT[:, dt, :].bitcast(F32))

                with tc.tile_pool(name="eps_", bufs=2, space="PSUM") as pp:
                    wq2_sb = wmain.tile([P, DT, D], BF16, tag="w2m", bufs=2)
                    nc.scalar.dma_start(wq2_sb, wview(wq2T))
                    hold2 = {}

                    def pre2():
                        hold2["rstd"] = bcast_rstd(pp, rstd2_row)

                    proj(
                        pp, wq2_sb, xs2,
                        lambda oc, ps: nc.vector.scalar_tensor_tensor(
                            out=Q2_sb[:, oc, :], in0=ps, scalar=SX,
                            in1=hold2["rstd"], op0=OP.mult, op1=OP.mult),
                        fixup=(ns_sb["q2"], mu2), tag="ppq2", bufs=8,
                        pre_evac=pre2)

                with tc.tile_pool(name="aps2", bufs=1, space="PSUM") as apsum, \
                     tc.tile_pool(name="asb2", bufs=1) as asb:
                    for j in range(DT):              # head pair j
                        # separate PSUM banks per head: the two score MMs hit
                        # disjoint PE row groups and run concurrently, so they
                        # must not share a PSUM bank write port.
                        s_ps0 = apsum.tile([CN, TO], F32, tag="s_ps", bufs=4)
                        s_ps1 = apsum.tile([CN, TO], F32, tag="s_ps", bufs=4)
                        for hh, sp in ((0, s_ps0), (1, s_ps1)):
                            r0 = hh * 64
                            nc.tensor.matmul(
                                sp, K2_sb[r0:r0 + 64, j, :],
                                Q2_sb[r0:r0 + 64, j, :],
                                start=True, stop=True)
                        e_t = asb.tile([CN, 2, TO], FP8, tag="e_t", bufs=4)
                        nc.scalar.activation(e_t[:, 0, :], s_ps0, AF.Exp,
                                             scale=SCALE / (SX * SX))
                        nc.scalar.activation(e_t[:, 1, :], s_ps1, AF.Exp,
                                             scale=SCALE / (SX * SX))
                        o_ps0 = apsum.tile([65, TO], F32, tag="o_ps", bufs=4)
                        o_ps1 = apsum.tile([65, TO], F32, tag="o_ps", bufs=4)
                        nc.tensor.matmul(o_ps0, V2_sb[0:CN, 2 * j, :],
                                         e_t[:, 0, :], start=True, stop=True)
                        nc.tensor.matmul(o_ps1, V2_sb[0:CN, 2 * j + 1, :],
                                         e_t[:, 1, :], start=True, stop=True)
                        h0, h1 = 2 * j, 2 * j + 1
                        nc.vector.tensor_copy(out=Ou2_sb[0:64, j, :],
                                              in_=o_ps0[0:64, :])
                        nc.vector.tensor_copy(
                            out=den2_sb[(h0 % 4) * 32:(h0 % 4) * 32 + 1,
                                        h0 // 4, :],
                            in_=o_ps0[64:65, :])
                        nc.vector.tensor_copy(out=Ou2_sb[64:128, j, :],
                                              in_=o_ps1[0:64, :])
                        nc.vector.tensor_copy(
                            out=den2_sb[(h1 % 4) * 32:(h1 % 4) * 32 + 1,
                                        h1 // 4, :],
                            in_=o_ps1[64:65, :])
                    rec2_sb = asb.tile([P, 4, TO], F32R, tag="rec", bufs=1)
                    rscr2 = asb.tile([P, 4, TO], F32, tag="rscr", bufs=1)
                    rec2_f = asb.tile([P, 4, TO], F32, tag="recf", bufs=1)
                    nc.vector.reciprocal_approx_accurate(rec2_f, den2_sb,
                                                         rscr2)
                    with nc.allow_low_precision("f32r round of recip"):
                        nc.vector.tensor_copy(out=rec2_sb, in_=rec2_f)
                    for h in range(H):
                        j, r0 = h >> 1, (h & 1) * 64
                        b, s = (h % 4) * 32, h // 4
                        if b == 96:
                            rfix = asb.tile([1, TO], F32R, tag="rfix",
                                            bufs=4)
                            nc.vector.tensor_copy(out=rfix,
                                                  in_=rec2_sb[96:97, s, :])
                            lhs_ap, rhs_ap = ones2[0:1, :], rfix
                        else:
                            lhs_ap = ones2[b:b + 1, :]
                            rhs_ap = rec2_sb[b:b + 1, s, :]
                        r_ps = apsum.tile([64, TO], F32, tag="o_ps", bufs=4)
                        nc.tensor.matmul(r_ps, lhs_ap, rhs_ap,
                                         start=True, stop=True)
                        nc.vector.tensor_tensor(
                            out=O2_sb[r0:r0 + 64, j, :],
                            in0=Ou2_sb[r0:r0 + 64, j, :],
                            in1=r_ps, op=OP.mult)

                with tc.tile_pool(name="eps2", bufs=3, space="PSUM") as pp:
                    wo2_sb = wmain.tile([P, DT, D], BF16, tag="w2m", bufs=2)
                    nc.scalar.dma_start(wo2_sb, wview(wo2T))

                    def add_residual2(oc, ps):
                        nc.vector.tensor_tensor(
                            out=x_ownT[:, oc, :],
                            in0=x_ownT[:, oc, :].bitcast(F32),
                            in1=ps, op=OP.add)

                    proj(pp, wo2_sb, O2_sb, add_residual2)
            nc.leave_named_scope("phE_xattn", scopeE[0], False)

            # ========== FFN scope: phase F ==========
            scopeF = nc.enter_named_scope("phF_ffn", False)
            with tc.tile_pool(name="cf", bufs=1) as cf:
                xs3 = cf.tile([P, DT, TO], BF16)
                Hbuf = cf.tile([P, FT, TO], BF16)

                mu3, rstd3_row, mu3_row = stats(
                    lambda dt: x_ownT[:, dt, :], "ln3")
                with tc.tile_pool(name="bc3", bufs=1, space="PSUM") as bp3:
                    rstd3_sb = bcast_rstd(bp3, rstd3_row)
                    mu3_ps = bp3.tile([P, TO], F32, tag="s_bc2", bufs=1)
                    nc.tensor.matmul(mu3_ps, ones_row, mu3_row,
                                     start=True, stop=True)
                    mu3_sb = statp.tile([P, TO], F32, tag="st_mub", bufs=1)
                    nc.vector.tensor_copy(out=mu3_sb, in_=mu3_ps)
                # full LN on the prescale so the GEGLU needs NO per-tile
                # mean-fixup matmuls (the FFN is PE-bound; 64 contract-1
                # matmuls were ~12us of TensorE time)
                for dt in range(DT):
                    tmp = statp.tile([P, TO], F32, tag="st_tmp", bufs=3)
                    eng1 = nc.gpsimd if dt % 2 else nc.vector
                    eng1.tensor_tensor(out=tmp,
                                       in0=x_ownT[:, dt, :].bitcast(F32),
                                       in1=mu3_sb, op=OP.subtract)
                    eng2 = nc.vector if dt % 2 else nc.gpsimd
                    eng2.tensor_tensor(out=xs3[:, dt, :], in0=tmp,
                                       in1=rstd3_sb, op=OP.mult)

                wgT_v = wview(wgT)
                wfT_v = wfT.rearrange("(fp two p) o -> p fp two o",
                                      two=2, p=P)
                with tc.tile_pool(name="wg", bufs=1) as wgpool, \
                     tc.tile_pool(name="wfp", bufs=1) as wfpool, \
                     tc.tile_pool(name="gps", bufs=1, space="PSUM") as gpsum, \
                     tc.tile_pool(name="yps", bufs=1, space="PSUM") as ypool, \
                     tc.tile_pool(name="gsb", bufs=3) as gsb:
                    i_ps = ypool.tile([P, DT, TO], F32)

                    def ffout_block(fp, first, last):
                        wf_t = wfpool.tile([P, 2, D], BF16, tag="wft",
                                           bufs=6)
                        eng = nc.sync if fp % 2 == 0 else nc.gpsimd
                        eng.dma_start(wf_t, wfT_v[:, fp])
                        for oc in range(DT):
                            for i in range(2):
                                # oc pairs share a 2KB PSUM bank: only the
                                # even oc's first matmul may start=True
                                # (bank-wide has_written clear); the odd
                                # oc's first write lands on cleared bits and
                                # overwrites.
                                nc.tensor.matmul(
                                    i_ps[:, oc, :],
                                    wf_t[:, i, oc * P:(oc + 1) * P],
                                    Hbuf[:, 2 * fp + i, :],
                                    start=(first and i == 0 and oc % 2 == 0),
                                    stop=(last and i == 1),
                                    skip_group_check=(first and i == 0
                                                      and oc % 2 == 1))

                    for g in range(8):
                        wg_h = wgpool.tile([P, DT, 512], BF16, tag="wgh",
                                           bufs=2)
                        nc.sync.dma_start(wg_h,
                                          wgT_v[:, :, g * 512:(g + 1) * 512])
                        wg_g = wgpool.tile([P, DT, 512], BF16, tag="wgg",
                                           bufs=2)
                        nc.gpsimd.dma_start(
                            wg_g, wgT_v[:, :, FF + g * 512:FF + (g + 1) * 512])
                        for fi2 in range(2):
                            # two f-tiles (h and gate halves) share PSUM
                            # banks so gelu runs over [128, 512]
                            h_ps = gpsum.tile([P, 2, TO], F32, tag="h_ps",
                                              bufs=2)
                            g_ps = gpsum.tile([P, 2, TO], F32, tag="g_ps",
                                              bufs=2)
                            for k in range(2):
                                fi = fi2 * 2 + k
                                for it in range(DT):
                                    nc.tensor.matmul(
                                        h_ps[:, k, :],
                                        wg_h[:, it, fi * P:(fi + 1) * P],
                                        xs3[:, it, :],
                                        start=(k == 0 and it == 0),
                                        stop=(it == DT - 1),
                                        skip_group_check=(k == 1 and it == 0))
                                for it in range(DT):
                                    nc.tensor.matmul(
                                        g_ps[:, k, :],
                                        wg_g[:, it, fi * P:(fi + 1) * P],
                                        xs3[:, it, :],
                                        start=(k == 0 and it == 0),
                                        stop=(it == DT - 1),
                                        skip_group_check=(k == 1 and it == 0))
                            gel = gsb.tile([P, 2, TO], F32, tag="gel",
                                           bufs=3)
                            nc.scalar.activation(gel, g_ps, AF.Gelu)
                            f0 = g * 4 + fi2 * 2
                            nc.vector.tensor_tensor(
                                out=Hbuf[:, f0:f0 + 2, :],
                                in0=h_ps, in1=gel, op=OP.mult)

                        # ffout for the weight pairs whose H tiles completed
                        # in the previous g-block (pipelined)
                        if g >= 1:
                            ffout_block(2 * (g - 1), first=(g == 1), last=False)
                            ffout_block(2 * (g - 1) + 1, first=False,
                                        last=False)
                    ffout_block(14, first=False, last=False)
                    ffout_block(15, first=False, last=True)
                    for oc in range(DT):
                        nc.vector.tensor_tensor(
                            out=x_ownT[:, oc, :],
                            in0=x_ownT[:, oc, :].bitcast(F32),
                            in1=i_ps[:, oc, :], op=OP.add)
                        nc.sync.dma_start(yT_v[:, oc, :], x_ownT[:, oc, :])
            nc.leave_named_scope("phF_ffn", scopeF[0], False)

    nc.finalize()
    return nc


_CACHE = {}


def kernel(**inputs):
    def f32c(a):
        return np.ascontiguousarray(np.asarray(a, dtype=np.float32))

    bf16 = ml_dtypes.bfloat16

    def bfT(w):
        """W [out,in] (optionally gain-folded) -> bf16 W.T contiguous."""
        return np.ascontiguousarray(w.T).astype(bf16)

    def nrow(w):
        """-colsum over the input dim, as a [1, out] bf16 row."""
        return (-w.sum(axis=1))[None, :].astype(bf16)

    ISX = 1.0 / SX
    x = f32c(inputs["hidden_states"])[0]          # [N, D]
    ctx = f32c(inputs["context"])[0]              # [CN, CD]
    g1 = f32c(inputs["ln1_g"]); b1 = f32c(inputs["ln1_b"])
    g2 = f32c(inputs["ln2_g"]); b2 = f32c(inputs["ln2_b"])
    g3 = f32c(inputs["ln3_g"]); b3 = f32c(inputs["ln3_b"])
    wq1 = f32c(inputs["wq1"]); wk1 = f32c(inputs["wk1"]); wv1 = f32c(inputs["wv1"])
    wo1 = f32c(inputs["wo1"]); bo1 = f32c(inputs["bo1"])
    wq2 = f32c(inputs["wq2"]); wk2 = f32c(inputs["wk2"]); wv2 = f32c(inputs["wv2"])
    wo2 = f32c(inputs["wo2"]); bo2 = f32c(inputs["bo2"])
    wg = f32c(inputs["w_geglu"]); bg = f32c(inputs["b_geglu"])
    wf = f32c(inputs["w_ffout"]); bf = f32c(inputs["b_ffout"])

    # this kernel folds LN affine gains into the weights; biases of the
    # reference setup are all zero
    for nm, b in (("ln1_b", b1), ("ln2_b", b2), ("ln3_b", b3),
                  ("bo1", bo1), ("bo2", bo2), ("b_geglu", bg),
                  ("b_ffout", bf)):
        assert not np.any(b), f"nonzero bias {nm} unsupported"

    if "nc" not in _CACHE:
        _CACHE["nc"] = build()
    nc = _CACHE["nc"]

    wq1g = wq1 * g1[None, :]
    wk1g = wk1 * g1[None, :]
    wv1g = wv1 * g1[None, :]
    wq2g = wq2 * g2[None, :]
    wgg = wg * g3[None, :]

    xT = np.ascontiguousarray(x.T)                # [D, N]
    shared = {
        "ctxT": np.ascontiguousarray(ctx.T).astype(bf16),
        "wq1T": bfT(wq1g), "wk1T": bfT(wk1g), "wv1T": bfT(wv1g),
        "wo1T": bfT(wo1 * ISX),
        "wq2T": bfT(wq2g),
        "wk2T": bfT(wk2), "wv2T": bfT(wv2),
        "wo2T": bfT(wo2 * ISX),
        "wgT": bfT(wgg), "wfT": bfT(wf),
        "nsk1": nrow(wk1g), "nsq1": nrow(wq1g), "nsv1": nrow(wv1g),
        "nsq2": nrow(wq2g),
        "onesc": np.ones((P, 1), np.float32),
        "onesr": np.ones((1, P), np.float32),
    }

    in_maps = []
    for c in range(NCORES):
        m = dict(shared)
        xc = np.ascontiguousarray(xT[:, c * TO:(c + 1) * TO])
        m["xoT"] = xc
        m["xobT"] = xc.astype(bf16)
        in_maps.append(m)

    res = run_bass_kernel_spmd(nc, in_maps, core_ids=list(range(NCORES)))
    yT = np.concatenate([r["yT"] for r in res.results], axis=1)  # [D, N]
    return np.ascontiguousarray(yT.T)[None].astype(np.float32)
